# revision 14
# baseline (speedup 1.0000x reference)
"""MiniMax MoE gate (sigmoid + bias, top-8, normalized weights) on 8 TRN2 cores.

Full inputs in, full outputs out. Data-parallel over tokens: each core gets
1024 of the 8192 tokens; gate weight and bias are replicated.

Strategy:
  - The host pre-transposes x and W and splits them into bf16 hi/lo pairs
    (x == hi + lo to within ~2^-18 relative). logits are accumulated in fp32
    PSUM as the 3-term compensated product x_hi@W_hi + x_hi@W_lo + x_lo@W_hi,
    which lands at fp32-level accuracy (validated: identical top-8 flip count
    to a full fp32 kernel). bf16 operands run the PE at 1 cycle/row with fast
    weight loads; shipping x^T removes all on-chip transposes.
  - The host lays x^T out so every DMA is CONTIGUOUS PER PARTITION in DRAM
    (kilobyte descriptors), x-hi rides the SP HWDGE queue and x-lo the
    Activation HWDGE queue (the two hardware queues run in parallel), and all
    x DMA triggers are queued up front so no compute op ever blocks a
    descriptor-generation sequencer.
  - Tokens are processed in groups with a [3,2,2,1]x128 plan: a big first
    group (PE gets work early), a small last group (short exposed tail). The
    first group's loads are split fine ([4,4,8,16] chunks) so the first
    matmul can start within a few microseconds; steady-state loads use 8-16KB
    descriptors for full HBM rate.
  - Tokens are interleaved within each group (output partition q of tile i
    holds token q*ntg + i) so the [ts, 8] outputs are also per-partition
    contiguous: two output DMAs per group.
  - x^T hi/lo chunk tiles are the matmul stationary operand (fast weight
    load), W^T chunks the moving operand; logits come out [128 tok x 64 exp],
    the layout the top-8 epilogue needs.
  - sigmoid = 1/(1 + e^-l) via ACT exp (~2 ULP) + DVE reciprocal; top-8 via
    DVE MAX8/FIND_INDEX8 (descending, ties by ascending index - exactly
    jax.lax.top_k); per-slot raw scores via one fused scalar_tensor_tensor
    per slot; weights = ssel * recip(sum).
"""

import os

import numpy as np
import ml_dtypes

import concourse.bacc as bacc
import concourse.mybir as mybir
from concourse.bass_utils import run_bass_kernel_spmd
from concourse.tile import TileContext

T, D, E, K = 8192, 4096, 64, 8
NCORES = 8
P = 128
F32 = mybir.dt.float32
BF16 = mybir.dt.bfloat16
BF16_NP = ml_dtypes.bfloat16
DC = D // P   # 32 contraction chunks

# tiles (x128 tokens) per token-group, per core
GROUP_PLAN = tuple(
    int(v) for v in os.environ.get("KPLAN", "3,2,2,1").split(",")
)
# chunk-split of each group's x loads (first group fine for fast startup)
SUB_PLAN_FIRST = (4, 4, 8, 16)
SUB_PLAN_REST = (16, 16)
SIGMOID_MODE = os.environ.get("KSIG", "exp")  # "exp" (precise) | "lut"


def _plans(ts):
    nt = ts // P
    plan = list(GROUP_PLAN)
    if sum(plan) != nt:  # fallback for small test shards
        plan = [1] * nt
    subs = [SUB_PLAN_FIRST if h == 0 else SUB_PLAN_REST for h in range(len(plan))]
    return plan, subs


def build_nc(ts):
    """Per-core program for a shard of `ts` tokens."""
    plan, subs = _plans(ts)
    nh = len(plan)
    ths = [p * P for p in plan]
    total_cols = DC * sum(ths)

    nc = bacc.Bacc("TRN2", target_bir_lowering=False)
    # host-tiled layout (see prepare_in_maps): group blocks side by side;
    # within a group, row p holds all of partition p's data contiguously.
    xhd = nc.dram_tensor("xt_hi", [P, total_cols], BF16, kind="ExternalInput")
    xld = nc.dram_tensor("xt_lo", [P, total_cols], BF16, kind="ExternalInput")
    whd = nc.dram_tensor("wt_hi", [P, DC * E], BF16, kind="ExternalInput")
    wld = nc.dram_tensor("wt_lo", [P, DC * E], BF16, kind="ExternalInput")
    b = nc.dram_tensor("bias", [1, E], F32, kind="ExternalInput")
    oi = nc.dram_tensor("out_idx", [ts, K], mybir.dt.int32, kind="ExternalOutput")
    ow = nc.dram_tensor("out_w", [ts, K], F32, kind="ExternalOutput")

    with TileContext(nc) as tc:
        with (
            tc.tile_pool(name="const", bufs=1) as cpool,
            tc.tile_pool(name="xin", bufs=1) as xpool,
            tc.tile_pool(name="epi", bufs=2) as epool,
            tc.tile_pool(name="outb", bufs=2) as opool,
            tc.tile_pool(name="plogit", bufs=2, space="PSUM") as plogit,
        ):
            bias_row = cpool.tile([1, E], F32)
            nc.gpsimd.dma_start(out=bias_row, in_=b[:, :])
            bias_bc = cpool.tile([P, E], F32)
            nc.gpsimd.partition_broadcast(bias_bc, bias_row)

            # W via the software DGE so both hardware queues start on x
            wth = cpool.tile([P, DC, E], BF16)
            nc.gpsimd.dma_start(out=wth, in_=whd[:, :])
            wtl = cpool.tile([P, DC, E], BF16)
            nc.gpsimd.dma_start(out=wtl, in_=wld[:, :])

            # queue ALL x loads up front (hi -> SP queue, lo -> ACT queue)
            xhts, xlts = [], []
            off = 0
            for h in range(nh):
                th = ths[h]
                xht = xpool.tile([P, DC, th], BF16, tag=f"xh{h}", name=f"xh{h}")
                xlt = xpool.tile([P, DC, th], BF16, tag=f"xl{h}", name=f"xl{h}")
                c0 = 0
                for cs in subs[h]:
                    sl = slice(c0 * th, (c0 + cs) * th)
                    dsl = slice(off + c0 * th, off + (c0 + cs) * th)
                    nc.sync.dma_start(
                        out=xht[:].rearrange("p c t -> p (c t)")[:, sl],
                        in_=xhd[:, dsl],
                    )
                    nc.scalar.dma_start(
                        out=xlt[:].rearrange("p c t -> p (c t)")[:, sl],
                        in_=xld[:, dsl],
                    )
                    c0 += cs
                off += DC * th
                xhts.append(xht)
                xlts.append(xlt)

            tok0 = 0
            for h in range(nh):
                th, ntg = ths[h], plan[h]
                xht, xlt = xhts[h], xlts[h]
                lgs = [
                    plogit.tile([P, E], F32, tag=f"lg{i}", name=f"lg_h{h}_{i}")
                    for i in range(ntg)
                ]
                for c in range(DC):
                    for i, lg in enumerate(lgs):
                        tsl = slice(i * P, (i + 1) * P)
                        nc.tensor.matmul(
                            lg, xht[:, c, tsl], wth[:, c, :],
                            start=(c == 0), stop=False,
                        )
                        nc.tensor.matmul(
                            lg, xht[:, c, tsl], wtl[:, c, :],
                            start=False, stop=False,
                        )
                        nc.tensor.matmul(
                            lg, xlt[:, c, tsl], wth[:, c, :],
                            start=False, stop=(c == DC - 1),
                        )

                oidx = opool.tile(
                    [P, ntg, K], mybir.dt.uint32, tag=f"oidx{ntg}", name=f"oidx{h}"
                )
                owgt = opool.tile(
                    [P, ntg, K], F32, tag=f"owgt{ntg}", name=f"owgt{h}"
                )
                for i, lg in enumerate(lgs):
                    sc = epool.tile([P, E], F32, tag="sc")
                    if SIGMOID_MODE == "lut":
                        nc.scalar.activation(
                            out=sc, in_=lg,
                            func=mybir.ActivationFunctionType.Sigmoid,
                        )
                    else:
                        # sigmoid = 1 / (1 + e^-l)
                        ex = epool.tile([P, E], F32, tag="ex")
                        nc.scalar.activation(
                            out=ex, in_=lg,
                            func=mybir.ActivationFunctionType.Exp, scale=-1.0,
                        )
                        den = epool.tile([P, E], F32, tag="den")
                        nc.vector.tensor_scalar_add(den, ex, 1.0)
                        nc.vector.reciprocal(out=sc, in_=den)
                    bi = epool.tile([P, E], F32, tag="bi")
                    nc.vector.tensor_tensor(
                        out=bi, in0=sc, in1=bias_bc, op=mybir.AluOpType.add
                    )
                    msel = epool.tile([P, K], F32, tag="msel")
                    nc.vector.max(out=msel, in_=bi)
                    nc.vector.max_index(
                        out=oidx[:, i, :], in_max=msel, in_values=bi
                    )
                    ssel = epool.tile([P, K], F32, tag="ssel")
                    for k in range(K):
                        scr = epool.tile([P, E], F32, tag="scr")
                        nc.vector.scalar_tensor_tensor(
                            out=scr,
                            in0=bi,
                            scalar=msel[:, k:k + 1],
                            in1=sc,
                            op0=mybir.AluOpType.is_equal,
                            op1=mybir.AluOpType.mult,
                            accum_out=ssel[:, k:k + 1],
                        )
                    ssum = epool.tile([P, 1], F32, tag="ssum")
                    nc.vector.tensor_reduce(
                        out=ssum, in_=ssel,
                        axis=mybir.AxisListType.X, op=mybir.AluOpType.add,
                    )
                    rsum = epool.tile([P, 1], F32, tag="rsum")
                    nc.vector.reciprocal(out=rsum, in_=ssum)
                    nc.vector.tensor_scalar_mul(owgt[:, i, :], ssel, rsum[:])

                # token at output partition q of tile i is tok0 + q*ntg + i
                nc.sync.dma_start(
                    out=oi[tok0:tok0 + th, :].rearrange(
                        "(q i) k -> q i k", i=ntg
                    ),
                    in_=oidx[:].bitcast(mybir.dt.int32),
                )
                nc.sync.dma_start(
                    out=ow[tok0:tok0 + th, :].rearrange(
                        "(q i) k -> q i k", i=ntg
                    ),
                    in_=owgt,
                )
                tok0 += th

    nc.compile()
    return nc


_NC_CACHE = {}


def _get_nc(ts):
    if ts not in _NC_CACHE:
        _NC_CACHE[ts] = build_nc(ts)
    return _NC_CACHE[ts]


def _tile_xt(xs, ts):
    """[ts, D] fp32 -> [P, DC*ts] fp32 in the device layout.

    Groups laid side by side; within group h (tiles ntg, tokens th=128*ntg),
    flat column off_h + c*th + i*P + q holds x[tok0 + q*ntg + i, c*P + p]
    at partition row p.
    """
    plan, _ = _plans(ts)
    blocks = []
    tok0 = 0
    for ntg in plan:
        th = ntg * P
        a = xs[tok0:tok0 + th].reshape(P, ntg, DC, P)  # [q, i, c, p]
        a = a.transpose(3, 2, 1, 0)                    # [p, c, i, q]
        blocks.append(np.ascontiguousarray(a).reshape(P, DC * th))
        tok0 += th
    return np.concatenate(blocks, axis=1)


def prepare_in_maps(x, gate_weight, bias):
    x = np.asarray(x, dtype=np.float32)
    gw = np.asarray(gate_weight, dtype=np.float32)
    bb = np.ascontiguousarray(np.asarray(bias, dtype=np.float32)).reshape(1, E)

    ts = T // NCORES

    # W^T in device layout [P, DC*E]: [p, c*E + e] = W[e, c*P + p]
    wt = np.ascontiguousarray(gw.T.reshape(DC, P, E).transpose(1, 0, 2)).reshape(
        P, DC * E
    )
    wh = wt.astype(BF16_NP)
    wl = (wt - wh.astype(np.float32)).astype(BF16_NP)

    in_maps = []
    for cid in range(NCORES):
        xt = _tile_xt(x[cid * ts:(cid + 1) * ts], ts)
        xh = xt.astype(BF16_NP)
        xl = (xt - xh.astype(np.float32)).astype(BF16_NP)
        in_maps.append({
            "xt_hi": xh,
            "xt_lo": xl,
            "wt_hi": wh,
            "wt_lo": wl,
            "bias": bb,
        })
    return in_maps


def kernel(x, gate_weight, bias):
    ts = T // NCORES
    nc = _get_nc(ts)
    in_maps = prepare_in_maps(x, gate_weight, bias)
    res = run_bass_kernel_spmd(nc, in_maps, core_ids=list(range(NCORES)))
    idx = np.concatenate([r["out_idx"] for r in res.results], axis=0)
    wts = np.concatenate([r["out_w"] for r in res.results], axis=0)
    return idx, wts
